# revision 9
# baseline (speedup 1.0000x reference)
"""DistMult edge scoring on 8 Trainium2 NeuronCores.

    score[r, e] = sigmoid( sum_d h[src[r,e], d] * W[r, d] * h[dst[r,e], d] )

Strategy (edge-parallel, h replicated — per sharding hint):
  - Edges sharded across 8 cores (contiguous 25000-edge slice per relation).
  - h rows are gathered from HBM with the SWDGE dma_gather instruction.
    dma_gather takes int16 indices, so nodes are split into 4 chunks of
    25000 rows; each core's edges are bucketed by (relation, src_chunk,
    dst_chunk) on the host into fixed 1664-position slots (trailing -1
    indices are trimmed by the Q7 ucode, so slot padding costs nothing).
  - Per bucket: gather hu/hv tiles [128 edges x 128 d], multiply hv by the
    per-relation W row (vector engine, step-0 broadcast AP), multiply by hu,
    then reduce each 128-edge tile on the scalar engine via the ACTIVATE
    free-dim accumulator (tensor_tensor_reduce crashes this toolchain).
  - Sigmoid on the scalar engine, one output DMA, host un-permutes.
  - Rare buckets overflowing the 1664-edge slot are computed on the host
    (numpy) and patched in (~0.1% of edges in expectation).
"""

import numpy as np

N_NODES = 100000
N_REL = 6
D = 128
E = 200000
M = 8  # cores

CH = 4  # node chunks (int16 index limit)
CHUNK = N_NODES // CH  # 25000
E_CORE = E // M  # edges per relation per core
NBUCK = N_REL * CH * CH  # 96 buckets per core

B_PAD = 1664  # bucket slot capacity (13 tiles of 128)
TB = B_PAD // 128  # 13 tiles per bucket
F_B = B_PAD // 16  # idx free-dim per bucket-side (104)
NCOL = NBUCK * TB  # score columns per core (1248)

NQUEUES = 4
SINGLE_PACKET = False

_NC_CACHE = {}


def _build_nc(nbuck_limit=None):
    import sys
    if "/opt/trn_rl_repo" not in sys.path:
        sys.path.insert(0, "/opt/trn_rl_repo")
    import concourse.bacc as bacc
    import concourse.bass as bass
    import concourse.tile as tile
    import concourse.mybir as mybir
    from concourse import library_config
    from concourse.tile_rust import add_dep_helper

    f32 = mybir.dt.float32
    i16 = mybir.dt.int16

    nc = bacc.Bacc("TRN2", num_swdge_queues=NQUEUES)
    h_dram = nc.dram_tensor("h", [N_NODES, D], f32, kind="ExternalInput")
    wb_dram = nc.dram_tensor("wb", [128, N_REL * D], f32, kind="ExternalInput")
    idx_dram = nc.dram_tensor("idx", [128, NBUCK * 2 * F_B], i16, kind="ExternalInput")
    out_dram = nc.dram_tensor("scores", [128, NCOL], f32, kind="ExternalOutput")

    with tile.TileContext(nc) as tc:
        with (
            tc.tile_pool(name="const", bufs=1) as cpool,
            tc.tile_pool(name="g", bufs=3) as gpool,
            tc.tile_pool(name="wv", bufs=2) as wpool,
            tc.tile_pool(name="scr", bufs=4) as spool,
        ):
            lib = nc.gpsimd.load_library(library_config.mlp)
            idx_sbuf = cpool.tile([128, NBUCK * 2 * F_B], i16)
            wb_sbuf = cpool.tile([128, N_REL * D], f32)
            scores = cpool.tile([128, NCOL], f32)
            sig = cpool.tile([128, NCOL], f32)
            nc.sync.dma_start(out=idx_sbuf[:], in_=idx_dram[:])
            nc.sync.dma_start(out=wb_sbuf[:], in_=wb_dram[:])

            first = True
            for b in range(NBUCK if nbuck_limit is None else nbuck_limit):
                r = b // (CH * CH)
                i = (b // CH) % CH
                j = b % CH
                hu = gpool.tile([128, TB, D], f32, tag="hu")
                hv = gpool.tile([128, TB, D], f32, tag="hv")
                g1 = nc.gpsimd.dma_gather(
                    out_ap=hu[:],
                    in_ap=h_dram[i * CHUNK:(i + 1) * CHUNK, :],
                    idxs_ap=idx_sbuf[:, (2 * b) * F_B:(2 * b + 1) * F_B],
                    num_idxs=B_PAD,
                    num_idxs_reg=B_PAD,
                    elem_size=D,
                    queue_num=(2 * b) % NQUEUES,
                    single_packet=SINGLE_PACKET,
                )
                g2 = nc.gpsimd.dma_gather(
                    out_ap=hv[:],
                    in_ap=h_dram[j * CHUNK:(j + 1) * CHUNK, :],
                    idxs_ap=idx_sbuf[:, (2 * b + 1) * F_B:(2 * b + 2) * F_B],
                    num_idxs=B_PAD,
                    num_idxs_reg=B_PAD,
                    elem_size=D,
                    queue_num=(2 * b + 1) % NQUEUES,
                    single_packet=SINGLE_PACKET,
                )
                if first:
                    add_dep_helper(g1.ins, lib.ins, sync=False, reason="lib first")
                    add_dep_helper(g2.ins, lib.ins, sync=False, reason="lib first")
                    first = False
                # wv = hv * W[r] (W row repeated along the TB tile dim via a
                # step-0 access pattern)
                ws = wb_sbuf[:, r * D:(r + 1) * D]
                w_bc = bass.AP(
                    ws.tensor, ws.offset,
                    [tuple(ws.ap[0]), (0, TB), tuple(ws.ap[1])],
                )
                wv = wpool.tile([128, TB, D], f32, tag="wv")
                nc.vector.tensor_tensor(
                    out=wv[:],
                    in0=hv[:],
                    in1=w_bc,
                    op=mybir.AluOpType.mult,
                )
                prod = wpool.tile([128, TB, D], f32, tag="prod")
                nc.vector.tensor_tensor(
                    out=prod[:],
                    in0=hu[:],
                    in1=wv[:],
                    op=mybir.AluOpType.mult,
                )
                for t in range(TB):
                    scr = spool.tile([128, D], f32, tag="scr")
                    nc.scalar.activation(
                        out=scr[:],
                        in_=prod[:, t, :],
                        func=mybir.ActivationFunctionType.Copy,
                        bias=0.0,
                        scale=1.0,
                        accum_out=scores[:, b * TB + t:b * TB + t + 1],
                    )
            nc.scalar.activation(
                out=sig[:],
                in_=scores[:],
                func=mybir.ActivationFunctionType.Sigmoid,
            )
            nc.sync.dma_start(out=out_dram[:], in_=sig[:])
    nc.compile()
    return nc


def _get_nc():
    if "nc" not in _NC_CACHE:
        _NC_CACHE["nc"] = _build_nc()
    return _NC_CACHE["nc"]


def _prep_core(src_c, dst_c):
    """Bucket one core's edges.

    src_c, dst_c: [N_REL, E_CORE] int arrays (node ids).
    Returns (idx_arr [128, NBUCK*2*F_B] int16, meta for unpacking).
    """
    # Pad slots gather row 0 of their chunk (index 0) rather than using the
    # Q7's trailing-negative trim: trimmed rows leave uninitialized SBUF
    # (garbage/NaN bit patterns) that poison the downstream multiplies and
    # sigmoid, which the runtime flags as numerical errors.
    idx_arr = np.zeros((128, NBUCK * 2 * F_B), dtype=np.int16)
    # per-bucket recovery info
    col_of_edge = np.empty(N_REL * E_CORE, dtype=np.int64)  # flat position in grid
    valid = np.zeros(N_REL * E_CORE, dtype=bool)
    overflow = []  # (r, e_local) indices computed on host

    for r in range(N_REL):
        s = src_c[r].astype(np.int64)
        d = dst_c[r].astype(np.int64)
        bucket = (s // CHUNK) * CH + (d // CHUNK)  # 0..15
        order = np.argsort(bucket, kind="stable")
        sb = bucket[order]
        # boundaries of each bucket in sorted order
        counts = np.bincount(sb, minlength=CH * CH)
        starts = np.concatenate([[0], np.cumsum(counts)[:-1]])
        s_loc = (s % CHUNK).astype(np.int16)
        d_loc = (d % CHUNK).astype(np.int16)
        for q in range(CH * CH):
            b = r * CH * CH + q
            n = counts[q]
            sel = order[starts[q]:starts[q] + n]
            if n > B_PAD:
                overflow_sel = sel[B_PAD:]
                overflow.append((r, overflow_sel))
                sel = sel[:B_PAD]
                n = B_PAD
            if n == 0:
                continue
            # wrapped int16 layout: index k -> partition k%16 (all 8 groups),
            # free k//16
            for side, loc in ((0, s_loc), (1, d_loc)):
                v = np.zeros(B_PAD, dtype=np.int16)
                v[:n] = loc[sel]
                w = v.reshape(F_B, 16).T  # [16, F_B]
                blk = (2 * b + side) * F_B
                idx_arr[:, blk:blk + F_B] = np.tile(w, (8, 1))
            # positions in the score grid
            k = np.arange(n)
            gcol = b * TB + k // 128
            grow = k % 128
            eflat = r * E_CORE + sel
            col_of_edge[eflat] = grow * NCOL + gcol  # encode (row, col)
            valid[eflat] = True
    return idx_arr, col_of_edge, valid, overflow


def kernel(h, W, src_idx, dst_idx):
    import sys
    if "/opt/trn_rl_repo" not in sys.path:
        sys.path.insert(0, "/opt/trn_rl_repo")
    from concourse.bass_utils import run_bass_kernel_spmd

    h = np.ascontiguousarray(np.asarray(h, dtype=np.float32))
    W = np.ascontiguousarray(np.asarray(W, dtype=np.float32))
    src = np.asarray(src_idx)
    dst = np.asarray(dst_idx)
    out_dtype = np.float32

    wb = np.tile(W.reshape(1, N_REL * D), (128, 1)).astype(np.float32)

    nc = _get_nc()
    in_maps = []
    metas = []
    for c in range(M):
        sl = slice(c * E_CORE, (c + 1) * E_CORE)
        idx_arr, col_of_edge, valid, overflow = _prep_core(src[:, sl], dst[:, sl])
        in_maps.append({"h": h, "wb": wb, "idx": idx_arr})
        metas.append((col_of_edge, valid, overflow))

    res = run_bass_kernel_spmd(nc, in_maps, core_ids=list(range(M)))

    out = np.empty((N_REL, E), dtype=out_dtype)
    for c in range(M):
        col_of_edge, valid, overflow = metas[c]
        sc = res.results[c]["scores"].reshape(-1)  # [128*NCOL] row-major
        flat = np.empty(N_REL * E_CORE, dtype=np.float32)
        flat[valid] = sc[col_of_edge[valid]]
        # host-patch overflow edges
        sl = slice(c * E_CORE, (c + 1) * E_CORE)
        src_c = src[:, sl]
        dst_c = dst[:, sl]
        for r, sel in overflow:
            hu = h[src_c[r][sel].astype(np.int64)]
            hv = h[dst_c[r][sel].astype(np.int64)]
            sco = np.einsum("ed,d,ed->e", hu, W[r], hv)
            flat[r * E_CORE + sel] = 1.0 / (1.0 + np.exp(-sco))
        out[:, sl] = flat.reshape(N_REL, E_CORE)
    return out
